# revision 41
# baseline (speedup 1.0000x reference)
"""Trainium2 Bass kernel for nn_ExpertTokenMLPLoRABlock.

Transformer block (LN -> MHA -> residual -> LN -> MLP+token-range-LoRA ->
residual), B=8 batch sharded one batch element per NeuronCore (pure data
parallel, no collectives).

Host-side prep folds the LayerNorm affine transforms into the weight
matrices (LN(x) = z*g + b with z standardized, so h@W = z@(diag(g)W) +
b@W), folds the attention scale into wq, pre-transposes the mask, and
casts weights to bf16. On chip each core computes, for its batch row:

  P1  LN1 standardize (token-major)        -> PE-transpose -> z1T (feat-major)
  P2  qT,kT = W^T z1 (feat-major), v (token-major, 65-col head blocks
      with an appended ones column)
  P3  per head: scoresT[sk,sq] = k^T q + I@maskT (PSUM accumulate),
      expT = Exp(scoresT) [no max-subtraction needed: |scores|<~10],
      PV: [v_h | 1]^T @ expT -> unnormalized attnT + sumexp row,
      then one reciprocal over all heads' sums, per-head broadcast via
      K=1 matmul, fused normalize; WO projection + residual in place
  P4  LN2 standardize -> z2T
  P5  hiddenT = gelu(W1^T z2 + b1 + LoRA1 on expert cols)  (feat-major)
  P6  out = W2^T hidden + b2 + LoRA2 on expert rows; residual in place
"""

import os
import sys

sys.path.insert(0, "/opt/trn_rl_repo")
os.environ.setdefault("MYCRO_LOCAL_CACHE", "1")

import numpy as np
import ml_dtypes

import concourse.bass as bass
import concourse.mybir as mybir
import concourse.tile as tile
from concourse.bass_utils import run_bass_kernel_spmd

BF16 = mybir.dt.bfloat16
F32 = mybir.dt.float32
AF = mybir.ActivationFunctionType

B, S, D = 8, 1024, 768
H, HD = 12, 64
DFF = 4 * D
R = 16
EPS = 1e-6
N_CORES = 8

_cache = {}
last_exec_time_ns = None


def _install_ntff_hook():
    """Optional: register the axon NTFF profiling hook (used only when
    KERNEL_PROFILE=1; the grading path never needs this)."""
    import types
    try:
        import antenv.axon_hooks  # noqa: F401
        return
    except ImportError:
        pass
    try:
        mod = types.ModuleType("antenv.axon_hooks")
        _state = {"hook": None}
        mod.set_axon_ntff_profile_hook = lambda h: _state.__setitem__("hook", h)
        mod.get_axon_ntff_profile_hook = lambda: _state["hook"]
        sys.modules["antenv.axon_hooks"] = mod
        import antenv
        antenv.axon_hooks = mod
        if "/root/.axon_site" not in sys.path:
            sys.path.insert(0, "/root/.axon_site")
        from trn_agent_boot.trn_boot import _ntff_profile_via_ctypes
        mod.set_axon_ntff_profile_hook(
            _ntff_profile_via_ctypes('/opt/axon/libaxon_pjrt.so'))
    except Exception:
        pass


def _dedup_ldweights(nc):
    """Delete an InstLdweights whose weights AP is identical to the
    previous one on the PE stream with no intervening array-clobbering
    instruction (only transpose-mode matmuls and other ldweights load the
    weight array; plain matmuls consume without clobbering). Any sync
    waits on the deleted load move to the next PE instruction (split into
    NoOps later by _split_multi_waits if needed)."""
    import bass_rust

    # Only loads from tensors that are write-once-then-read during their
    # matmul phase may dedup: pool-slot tiles (z1/z2 temps, exp tiles, ...)
    # get rewritten with new data at the same address, which a static AP
    # signature cannot see.
    immutable = ("wq_sb", "wk_sb", "wv_sb", "w1_sb", "w2_sb", "hid_",
                 "ident_", "ones_bf", "ones64", "v_sb", "kT_sb", "qT_sb",
                 "la1_sb", "la2_sb", "lb1_sb", "lb2_sb", "t1T_sb", "t2T_sb",
                 "attnP", "wo_sb", "z1T", "z2T")

    def sig(ldw):
        a = ldw.ins[0]
        r = repr(a)
        if not any(tok in r for tok in immutable):
            return None
        return r

    for f in nc.m.functions:
        for bb in f.blocks:
            out = []
            last_sig = None
            pending_waits = []
            changed = False
            for inst in bb.instructions:
                eng = inst.engine
                if eng in (mybir.EngineType.PE,):
                    if isinstance(inst, mybir.InstLdweights):
                        s = sig(inst)
                        upd = (inst.sync_info.on_update
                               if inst.sync_info else None)
                        if s is not None and s == last_sig and not upd:
                            if inst.sync_info and inst.sync_info.on_wait:
                                pending_waits.extend(inst.sync_info.on_wait)
                            changed = True
                            continue  # drop duplicate load
                        last_sig = s
                    elif isinstance(inst, mybir.InstMatmult):
                        if getattr(inst, "is_transpose", False):
                            last_sig = None
                    else:
                        last_sig = None
                    if pending_waits:
                        w = list(pending_waits)
                        if inst.sync_info and inst.sync_info.on_wait:
                            w = list(inst.sync_info.on_wait) + w
                        inst.sync_info = bass_rust.SyncInfo(
                            on_wait=w,
                            on_update=(list(inst.sync_info.on_update)
                                       if inst.sync_info
                                       and inst.sync_info.on_update else []))
                        pending_waits = []
                out.append(inst)
            if changed:
                bb.instructions = out
    return nc


def _split_multi_waits(nc):
    """This walrus build accepts at most one sync-wait per instruction;
    hoist extras onto preceding same-engine NoOps (engines execute their
    stream in order, so this preserves semantics)."""
    import bass_rust
    uid = 0
    for f in nc.m.functions:
        for bb in f.blocks:
            out = []
            changed = False
            for inst in bb.instructions:
                si = inst.sync_info
                waits = list(si.on_wait) if si and si.on_wait else []
                if len(waits) > 1:
                    for w in waits[:-1]:
                        uid += 1
                        nop = mybir.InstNoOp(name=f"I-waitsplit-{uid}")
                        nop.engine = inst.engine
                        nop.sync_info = bass_rust.SyncInfo(
                            on_wait=[w], on_update=[])
                        nc.register_instruction(nop, overwrite=True)
                        out.append(nop)
                    inst.sync_info = bass_rust.SyncInfo(
                        on_wait=[waits[-1]],
                        on_update=list(si.on_update) if si.on_update else [])
                    changed = True
                out.append(inst)
            if changed:
                bb.instructions = out
    return nc


def _ln_standardize(nc, pool, x_ap, out_ap, eps_sb):
    """out = (x - mean(x)) * rsqrt(var(x) + eps), stats over free dim."""
    p, d = x_ap.shape
    fmax = 256  # gcd(BN_STATS_FMAX=512, 768)
    nsub = d // fmax
    xg = x_ap.rearrange("p (s f) -> p s f", f=fmax)
    stats = pool.tile([128, nsub, 6], F32, tag="ln_stats")
    for s in range(nsub):
        nc.vector.bn_stats(out=stats[:p, s, :], in_=xg[:, s, :])
    mv = pool.tile([128, 2], F32, tag="ln_mv")
    nc.vector.bn_aggr(out=mv[:p], in_=stats[:p])
    # mv[:,1] = sqrt(var + eps) then reciprocal -> rstd
    nc.scalar.activation(out=mv[:p, 1:2], in_=mv[:p, 1:2], func=AF.Sqrt,
                         bias=eps_sb[:p], scale=1.0)
    nc.vector.reciprocal(out=mv[:p, 1:2], in_=mv[:p, 1:2])
    nc.vector.tensor_scalar(out=out_ap, in0=x_ap,
                            scalar1=mv[:p, 0:1], scalar2=mv[:p, 1:2],
                            op0=mybir.AluOpType.subtract,
                            op1=mybir.AluOpType.mult)


def _build(nbt: int):
    """Build the single-core program (SPMD across 8 cores)."""
    se = S - nbt            # expert token count (256)
    e0 = nbt                # expert range start in sq
    ST, DT, FT = S // 128, D // 128, DFF // 128  # 8, 6, 24
    ET0 = e0 // 128         # first expert s-tile (6)

    nc = bass.Bass()
    dp = nc.declare_dram_parameter
    x_d = dp("x", [S, D], F32, isOutput=False)
    maskT_d = dp("maskT", [S, S], BF16, isOutput=False)
    ident_d = dp("ident", [128, 128], BF16, isOutput=False)
    onesb_d = dp("ones_bf", [1, 128], BF16, isOutput=False)
    wq_d = dp("wq_r", [D, D], BF16, isOutput=False)
    wk_d = dp("wk_r", [D, D], BF16, isOutput=False)
    wv_d = dp("wv_r", [D, D], BF16, isOutput=False)
    bq_d = dp("bq_r", [128, DT], F32, isOutput=False)
    bk_d = dp("bk_r", [128, DT], F32, isOutput=False)
    bv_d = dp("bv_row", [1, D], BF16, isOutput=False)
    wo_d = dp("wo_r", [128, H // 2, D], BF16, isOutput=False)
    bo_d = dp("bo_row", [1, D], BF16, isOutput=False)
    w1_d = dp("w1_r", [D, DFF], BF16, isOutput=False)
    b1_d = dp("b1_r", [128, FT], F32, isOutput=False)
    la1_d = dp("la1_r", [D, R], BF16, isOutput=False)
    bt1_d = dp("bt1", [R, 1], F32, isOutput=False)
    lb1_d = dp("lb1_r", [R, DFF], BF16, isOutput=False)
    w2_d = dp("w2_r", [DFF, D], BF16, isOutput=False)
    b2_d = dp("b2_row", [1, D], BF16, isOutput=False)
    la2_d = dp("la2_r", [DFF, R], BF16, isOutput=False)
    lb2_d = dp("lb2_r", [R, D], BF16, isOutput=False)
    out_d = dp("out", [S, D], F32, isOutput=True)

    add = mybir.AluOpType.add
    mult = mybir.AluOpType.mult

    with tile.TileContext(nc) as tc:
        with tc.tile_pool(name="persist", bufs=1) as persist:
            # ---- persistent tiles ----
            x_sb = persist.tile([128, ST, D], F32)
            x_r = x_d.rearrange("(t p) d -> p t d", p=128)
            for t in range(ST):
                nc.sync.dma_start(out=x_sb[:, t, :], in_=x_r[:, t, :])
            ident = persist.tile([128, 128], BF16)
            nc.sync.dma_start(out=ident, in_=ident_d[:, :])
            ones_bf = persist.tile([1, 128], BF16)
            nc.sync.dma_start(out=ones_bf, in_=onesb_d[:, :])
            # all-ones f32 living on partition 64, so K=1 broadcast matmuls
            # can pair with the sumexp row (also on partition 64)
            ones64 = persist.tile([65, 128], BF16)
            nc.vector.memset(ones64[64:65, :], 1.0)
            bq_sb = persist.tile([128, DT], F32)
            nc.sync.dma_start(out=bq_sb, in_=bq_d[:, :])
            bk_sb = persist.tile([128, DT], F32)
            nc.sync.dma_start(out=bk_sb, in_=bk_d[:, :])
            bv_sb = persist.tile([1, D], BF16)
            nc.sync.dma_start(out=bv_sb, in_=bv_d[:, :])
            bo_sb = persist.tile([1, D], BF16)
            nc.sync.dma_start(out=bo_sb, in_=bo_d[:, :])
            b2_sb = persist.tile([1, D], BF16)
            nc.sync.dma_start(out=b2_sb, in_=b2_d[:, :])
            eps_sb = persist.tile([128, 1], F32)
            nc.vector.memset(eps_sb, EPS)
            # FC1 weights preloaded into persistent tiles; the DMAs are
            # issued after the QKV weight loads (see below) so they overlap
            # attention without delaying the QKV-critical transfers
            w1_sb = persist.tile([128, DT, DFF], BF16)
            la1_sb = persist.tile([128, DT, R], BF16)
            lb1_sb = persist.tile([R, DFF], BF16)
            bt1_sb = persist.tile([R, 1], F32)
            b1_sb = persist.tile([128, FT], F32)

            # ---- attention-lifetime tiles ----
            attnlife_cm = tc.tile_pool(name="attnlife", bufs=1)
            attnlife = attnlife_cm.__enter__()
            maskT_sb = attnlife.tile([128, ST, S], BF16)
            nc.sync.dma_start(out=maskT_sb,
                              in_=maskT_d.rearrange("(t p) s -> p t s", p=128))
            wo_sb = attnlife.tile([128, H // 2, D], BF16)
            nc.sync.dma_start(out=wo_sb, in_=wo_d[:, :, :])
            qT_sb = attnlife.tile([128, DT, S], BF16)
            kT_sb = attnlife.tile([128, DT, S], BF16)
            v_sb = attnlife.tile([128, ST, H * 65], BF16)

            # ================= P1 + P2: LN1, z1T, QKV =================
            with tc.tile_pool(name="qkv_sb", bufs=1) as qsb, \
                 tc.tile_pool(name="qkv_tmp", bufs=3) as qtmp, \
                 tc.tile_pool(name="qkv_st", bufs=4) as qst, \
                 tc.tile_pool(name="tr_ps", bufs=2, space="PSUM") as trps, \
                 tc.tile_pool(name="mm_ps", bufs=4, space="PSUM") as mmps:
                wq_sb = qsb.tile([128, DT, D], BF16)
                nc.sync.dma_start(out=wq_sb,
                                  in_=wq_d.rearrange("(t p) n -> p t n", p=128))
                wk_sb = qsb.tile([128, DT, D], BF16)
                nc.sync.dma_start(out=wk_sb,
                                  in_=wk_d.rearrange("(t p) n -> p t n", p=128))
                wv_sb = qsb.tile([128, DT, D], BF16)
                nc.sync.dma_start(out=wv_sb,
                                  in_=wv_d.rearrange("(t p) n -> p t n", p=128))
                z1T = qsb.tile([128, DT, S], BF16)

                # deferred FC1 preload transfers (overlap QKV/attention)
                nc.sync.dma_start(
                    out=w1_sb, in_=w1_d.rearrange("(t p) n -> p t n", p=128))
                nc.sync.dma_start(out=la1_sb,
                                  in_=la1_d.rearrange("(t p) r -> p t r",
                                                      p=128))
                nc.sync.dma_start(out=lb1_sb, in_=lb1_d[:, :])
                nc.sync.dma_start(out=bt1_sb, in_=bt1_d[:, :])
                nc.sync.dma_start(out=b1_sb, in_=b1_d[:, :])

                # ~6us of dummy matmuls during the initial x/weight DMAs so
                # the PE HAM clock-gate is already released (2.4 GHz) when
                # real work starts
                warm = mmps.tile([128, 128], F32, tag="mm")
                for i in range(64):
                    nc.tensor.matmul(warm[:, :], ident[:, :], ident[:, :],
                                     start=(i == 0), stop=(i == 63))

                for t in range(ST):
                    z1 = qtmp.tile([128, D], BF16, tag="z1")
                    _ln_standardize(nc, qst, x_sb[:, t, :], z1[:, :], eps_sb)
                    for d in range(DT):
                        trp = trps.tile([128, 128], BF16, tag="tr")
                        nc.tensor.transpose(trp[:, :],
                                            z1[:, d * 128:(d + 1) * 128],
                                            ident[:, :])
                        dst = z1T[:, d, t * 128:(t + 1) * 128]
                        if d % 2 == 0:
                            nc.vector.tensor_copy(out=dst, in_=trp[:, :])
                        else:
                            nc.scalar.copy(out=dst, in_=trp[:, :])

                # qT / kT (feature-major)
                for (w_sb, b_sb, o_sb) in ((wq_sb, bq_sb, qT_sb),
                                           (wk_sb, bk_sb, kT_sb)):
                    for td in range(DT):
                        ps0 = mmps.tile([128, 512], F32, tag="mm")
                        ps1 = mmps.tile([128, 512], F32, tag="mm")
                        pss = (ps0, ps1)
                        for k in range(DT):
                            for c in range(2):
                                cs = slice(c * 512, (c + 1) * 512)
                                nc.tensor.matmul(
                                    pss[c][:, :],
                                    w_sb[:, k, td * 128:(td + 1) * 128],
                                    z1T[:, k, cs],
                                    start=(k == 0), stop=(k == DT - 1))
                        for c in range(2):
                            cs = slice(c * 512, (c + 1) * 512)
                            nc.scalar.activation(
                                out=o_sb[:, td, cs], in_=pss[c][:, :],
                                func=AF.Identity, bias=b_sb[:, td:td + 1])

                # v (token-major, head blocks of 65 with ones col)
                for t in range(ST):
                    chunks = ((0, 512), (512, 256))
                    psv = [mmps.tile([128, cn], F32, tag="mm", name=f"psv{ci}")
                           for ci, (c0, cn) in enumerate(chunks)]
                    for k in range(DT):
                        for ci, (c0, cn) in enumerate(chunks):
                            nc.tensor.matmul(
                                psv[ci][:, :],
                                z1T[:, k, t * 128:(t + 1) * 128],
                                wv_sb[:, k, c0:c0 + cn],
                                start=(k == 0), stop=False)
                    for ci, (c0, cn) in enumerate(chunks):
                        nc.tensor.matmul(psv[ci][:, :], ones_bf[0:1, :],
                                         bv_sb[0:1, c0:c0 + cn],
                                         start=False, stop=True)
                        h0, hn = c0 // 64, cn // 64
                        dst = v_sb[:, t, :].rearrange(
                            "p (h c) -> p h c", c=65)[:, h0:h0 + hn, 0:64]
                        nc.vector.tensor_copy(
                            out=dst,
                            in_=psv[ci].rearrange("p (h c) -> p h c", c=64))
                    nc.vector.memset(
                        v_sb[:, t, :].rearrange(
                            "p (h c) -> p h c", c=65)[:, :, 64:65], 1.0)

            # ================= P3: attention =================
            with tc.tile_pool(name="at_sb", bufs=1) as asb, \
                 tc.tile_pool(name="at_tmp", bufs=2) as atmp, \
                 tc.tile_pool(name="at_exp", bufs=8) as aexp:
                # head PAIR tiles: even head on partitions 0-63, odd head on
                # 64-127 (moved there by an SBUF->SBUF DMA, which CAN cross
                # partitions) so the WO projection runs full-K=128 matmuls
                attnP = asb.tile([128, H // 2, S], BF16)
                # 1/sumexp rows live on partition 64 (same lane as the PV
                # psum's ones-row output — engines cannot shift partitions)
                recips = asb.tile([65, H, S], BF16)

                # heads are processed in pairs occupying PE row groups
                # 0-63 / 64-127 so their K=64 score matmuls (issued
                # back-to-back) run concurrently in the array. Each
                # (pair, sq-chunk) iteration's tail ops (ln/exp recips and
                # the attn stash casts, which must wait for the last PV
                # matmul) are DEFERRED into the middle of the next
                # iteration's stream — otherwise they drain the whole
                # ACT/DVE pipeline at each iteration boundary.
                with tc.tile_pool(name="sc_ps", bufs=4, space="PSUM") as scps, \
                     tc.tile_pool(name="pv_ps", bufs=2, space="PSUM") as pvps:
                    def emit_tail(hp, c, pvt):
                        cs = slice(c * 512, (c + 1) * 512)
                        for h in (2 * hp, 2 * hp + 1):
                            lntmp = atmp.tile([65, 512], F32, tag="lntmp")
                            nc.scalar.activation(out=lntmp[64:65, :],
                                                 in_=pvt[h][64:65, :],
                                                 func=AF.Ln)
                            nc.scalar.activation(out=recips[64:65, h, cs],
                                                 in_=lntmp[64:65, :],
                                                 func=AF.Exp, scale=-1.0)
                            if h % 2 == 0:
                                nc.vector.tensor_copy(
                                    out=attnP[0:64, hp, cs],
                                    in_=pvt[h][0:64, :])
                            else:
                                otmp = atmp.tile([64, 512], BF16,
                                                 tag="otmp")
                                nc.vector.tensor_copy(out=otmp[:, :],
                                                      in_=pvt[h][0:64, :])
                                nc.sync.dma_start(
                                    out=attnP[64:128, hp, cs],
                                    in_=otmp[:, :])
                        # normalize in the same deferred slot (borrows a
                        # scores psum slot for the broadcast)
                        bc = scps.tile([128, 512], F32, tag="sc")
                        nc.tensor.matmul(bc[0:64, :], ones64[64:65, 0:64],
                                         recips[64:65, 2 * hp, cs],
                                         start=True, stop=True)
                        nc.tensor.matmul(bc[64:128, :], ones64[64:65, 0:64],
                                         recips[64:65, 2 * hp + 1, cs],
                                         start=True, stop=True)
                        nc.vector.tensor_mul(out=attnP[:, hp, cs],
                                             in0=attnP[:, hp, cs],
                                             in1=bc[:, :])

                    pend = None
                    for hp in range(H // 2):
                        td = hp
                        heads = (2 * hp, 2 * hp + 1)
                        for c in range(2):
                            cs = slice(c * 512, (c + 1) * 512)
                            pv_a = pvps.tile([65, 512], F32, tag="pv0")
                            pv_b = pvps.tile([65, 512], F32, tag="pv1")
                            pvt = {heads[0]: pv_a, heads[1]: pv_b}
                            exf = {}

                            def emit_pv(tk):
                                for h in heads:
                                    nc.tensor.matmul(
                                        pvt[h][:, :],
                                        v_sb[:, tk, h * 65:(h + 1) * 65],
                                        exf[(h, tk)][:, :],
                                        start=(tk == 0), stop=(tk == ST - 1))

                            # software pipeline, PV lagging 2 tiles so its
                            # operands (ACT exp -> DVE mask-mul) are always
                            # ready and the PE never micro-stalls
                            for tk in range(ST):
                                tks = slice(tk * 128, (tk + 1) * 128)
                                scs = {}
                                for i, h in enumerate(heads):
                                    po = i * 64
                                    sc = scps.tile([128, 512], F32, tag="sc")
                                    scs[h] = sc
                                    nc.tensor.matmul(
                                        sc[:, :],
                                        kT_sb[po:po + 64, td, tks],
                                        qT_sb[po:po + 64, td, cs],
                                        start=True, stop=False)
                                # additive mask via identity matmuls (pair
                                # emitted back-to-back: the identical ident
                                # load dedups); keeps the exp->PV chain
                                # PE->ACT->PE with no DVE hop
                                for h in heads:
                                    nc.tensor.matmul(
                                        scs[h][:, :], ident[:, :],
                                        maskT_sb[:, tk, cs],
                                        start=False, stop=True)
                                for h in heads:
                                    ex = aexp.tile([128, 512], BF16,
                                                   tag="exp")
                                    nc.scalar.activation(out=ex[:, :],
                                                         in_=scs[h][:, :],
                                                         func=AF.Exp)
                                    exf[(h, tk)] = ex
                                if tk >= 2:
                                    emit_pv(tk - 2)
                                if tk == 2 and pend is not None:
                                    emit_tail(*pend)
                                    pend = None
                            emit_pv(ST - 2)
                            emit_pv(ST - 1)
                            pend = (hp, c, pvt)
                    emit_tail(*pend)

                # WO projection + residual (in place into x_sb)
                with tc.tile_pool(name="wo_ps", bufs=4, space="PSUM") as wops:
                    for t in range(ST):
                        for (c0, cn) in ((0, 512), (512, 256)):
                            ps = wops.tile([128, cn], F32, tag="wo")
                            for hp in range(H // 2):
                                nc.tensor.matmul(
                                    ps[:, :],
                                    attnP[:, hp, t * 128:(t + 1) * 128],
                                    wo_sb[:, hp, c0:c0 + cn],
                                    start=(hp == 0), stop=False)
                            nc.tensor.matmul(ps[:, :], ones_bf[0:1, :],
                                             bo_sb[0:1, c0:c0 + cn],
                                             start=False, stop=True)
                            xs = x_sb[:, t, c0:c0 + cn]
                            nc.vector.tensor_tensor(out=xs, in0=xs,
                                                    in1=ps[:, :], op=add)

            attnlife_cm.__exit__(None, None, None)

            # ================= P4/P5/P6: LN2 + MLP + LoRA =================
            with tc.tile_pool(name="mlp_sb", bufs=1) as msb, \
                 tc.tile_pool(name="mlp_tmp", bufs=3) as mtmp, \
                 tc.tile_pool(name="mlp_st", bufs=4) as mst:
                z2T = msb.tile([128, DT, S], BF16)
                t1T_sb = msb.tile([R, se], BF16)
                hid = msb.tile([128, FT, S], BF16)
                # FC2 weights get their own (non-reused) space so their DMAs
                # overlap FC1 compute instead of stalling at the FC2 boundary
                w2_sb = msb.tile([128, FT, D], BF16)
                nc.sync.dma_start(
                    out=w2_sb,
                    in_=w2_d.rearrange("(t p) n -> p t n", p=128))
                la2_sb = msb.tile([128, FT, R], BF16)
                nc.sync.dma_start(
                    out=la2_sb,
                    in_=la2_d.rearrange("(t p) r -> p t r", p=128))
                lb2_sb = msb.tile([R, D], BF16)
                nc.sync.dma_start(out=lb2_sb, in_=lb2_d[:, :])
                t2T_sb = msb.tile([R, se], BF16)

                with tc.tile_pool(name="tr2_ps", bufs=2, space="PSUM") as trps2, \
                     tc.tile_pool(name="fc1_ps", bufs=2, space="PSUM") as f1ps, \
                     tc.tile_pool(name="t1_ps", bufs=1, space="PSUM") as t1ps:
                    for t in range(ST):
                        z2 = mtmp.tile([128, D], BF16, tag="z2")
                        _ln_standardize(nc, mst, x_sb[:, t, :], z2[:, :],
                                        eps_sb)
                        for d in range(DT):
                            trp = trps2.tile([128, 128], BF16, tag="tr2")
                            nc.tensor.transpose(trp[:, :],
                                                z2[:, d * 128:(d + 1) * 128],
                                                ident[:, :])
                            dst = z2T[:, d, t * 128:(t + 1) * 128]
                            if d % 2 == 0:
                                nc.vector.tensor_copy(out=dst, in_=trp[:, :])
                            else:
                                nc.scalar.copy(out=dst, in_=trp[:, :])

                    # t1T = la1^T z2 (expert cols) + bt1
                    t1p = t1ps.tile([R, se], F32)
                    for k in range(DT):
                        nc.tensor.matmul(t1p[:, :], la1_sb[:, k, :],
                                         z2T[:, k, e0:S],
                                         start=(k == 0), stop=(k == DT - 1))
                    nc.scalar.activation(out=t1T_sb[:, :], in_=t1p[:, :],
                                         func=AF.Identity,
                                         bias=bt1_sb[:, 0:1])

                    # hiddenT = gelu(w1^T z2 + b1 [+ lb1^T t1 on expert cols])
                    for f in range(FT):
                        ps = f1ps.tile([128, S], F32, tag="fc1")
                        for k in range(DT):
                            for c in range(2):
                                cs = slice(c * 512, (c + 1) * 512)
                                nc.tensor.matmul(
                                    ps[:, cs],
                                    w1_sb[:, k, f * 128:(f + 1) * 128],
                                    z2T[:, k, cs],
                                    start=(k == 0),
                                    stop=(c == 0 and k == DT - 1),
                                    skip_group_check=True)
                        nc.tensor.matmul(ps[:, e0:S],
                                         lb1_sb[:, f * 128:(f + 1) * 128],
                                         t1T_sb[:, :],
                                         start=False, stop=True,
                                         skip_group_check=True)
                        nc.scalar.activation(out=hid[:, f, :], in_=ps[:, :],
                                             func=AF.Gelu,
                                             bias=b1_sb[:, f:f + 1])

                with tc.tile_pool(name="fc2_ps", bufs=4, space="PSUM") as f2ps, \
                     tc.tile_pool(name="t2_ps", bufs=1, space="PSUM") as t2ps:
                    # t2T = la2^T gelu(hidden) (expert cols)
                    t2p = t2ps.tile([R, se], F32)
                    for k in range(FT):
                        nc.tensor.matmul(t2p[:, :], la2_sb[:, k, :],
                                         hid[:, k, e0:S],
                                         start=(k == 0), stop=(k == FT - 1))
                    nc.vector.tensor_copy(out=t2T_sb[:, :], in_=t2p[:, :])

                    # out = w2^T hidden + b2 [+ t2 lb2 on expert rows] + x2
                    for t in range(ST):
                        chunks = ((0, 512), (512, 256))
                        psy = [f2ps.tile([128, cn], F32, tag="fc2",
                                         name=f"psy{ci}")
                               for ci, (c0, cn) in enumerate(chunks)]
                        for k in range(FT):
                            for ci, (c0, cn) in enumerate(chunks):
                                nc.tensor.matmul(
                                    psy[ci][:, :],
                                    hid[:, k, t * 128:(t + 1) * 128],
                                    w2_sb[:, k, c0:c0 + cn],
                                    start=(k == 0), stop=False)
                        last = t < ET0
                        for ci, (c0, cn) in enumerate(chunks):
                            nc.tensor.matmul(psy[ci][:, :], ones_bf[0:1, :],
                                             b2_sb[0:1, c0:c0 + cn],
                                             start=False, stop=last)
                        if t >= ET0:
                            for ci, (c0, cn) in enumerate(chunks):
                                nc.tensor.matmul(
                                    psy[ci][:, :],
                                    t2T_sb[:, (t - ET0) * 128:
                                           (t - ET0 + 1) * 128],
                                    lb2_sb[:, c0:c0 + cn],
                                    start=False, stop=True)
                        for ci, (c0, cn) in enumerate(chunks):
                            xs = x_sb[:, t, c0:c0 + cn]
                            nc.vector.tensor_tensor(out=xs, in0=xs,
                                                    in1=psy[ci][:, :], op=add)
                        nc.sync.dma_start(
                            out=out_d.rearrange("(t p) d -> p t d",
                                                p=128)[:, t, :],
                            in_=x_sb[:, t, :])

    if os.environ.get('DEDUP_LDW', '0') == '1':
        _dedup_ldweights(nc)  # unsafe on this walrus build; keep off
    _split_multi_waits(nc)
    return nc


def _prep_host(inputs):
    """Fold LN affines/attn-scale into weights; build DRAM-side layouts."""
    f32 = np.float32
    bf = ml_dtypes.bfloat16
    g1 = np.asarray(inputs["ln1_g"], f32)
    b1l = np.asarray(inputs["ln1_b"], f32)
    g2 = np.asarray(inputs["ln2_g"], f32)
    b2l = np.asarray(inputs["ln2_b"], f32)
    wq = np.asarray(inputs["wq"], f32)
    wk = np.asarray(inputs["wk"], f32)
    wv = np.asarray(inputs["wv"], f32)
    wo = np.asarray(inputs["wo"], f32)
    w1 = np.asarray(inputs["w1"], f32)
    w2 = np.asarray(inputs["w2"], f32)
    la1 = np.asarray(inputs["la1"], f32)
    lb1 = np.asarray(inputs["lb1"], f32)
    la2 = np.asarray(inputs["la2"], f32)
    lb2 = np.asarray(inputs["lb2"], f32)
    scale = HD ** -0.5

    def fold(g, bl, w, b):
        return (g[:, None] * w), (np.asarray(b, f32) + bl @ w)

    wq_f, bq_f = fold(g1, b1l, wq, inputs["bq"])
    wq_f, bq_f = wq_f * scale, bq_f * scale
    wk_f, bk_f = fold(g1, b1l, wk, inputs["bk"])
    wv_f, bv_f = fold(g1, b1l, wv, inputs["bv"])
    w1_f, b1_f = fold(g2, b2l, w1, inputs["b1"])
    la1_f = g2[:, None] * la1
    bt1 = b2l @ la1

    common = {
        "maskT": np.ascontiguousarray(
            np.asarray(inputs["attn_mask"], f32).T).astype(bf),
        "ident": np.eye(128, dtype=f32).astype(bf),
        "ones_bf": np.ones((1, 128), f32).astype(bf),
        "ones_f32": np.ones((1, 128), f32),
        "wq_r": wq_f.astype(bf),
        "wk_r": wk_f.astype(bf),
        "wv_r": wv_f.astype(bf),
        "bq_r": np.ascontiguousarray(bq_f.reshape(D // 128, 128).T),
        "bk_r": np.ascontiguousarray(bk_f.reshape(D // 128, 128).T),
        "bv_row": bv_f.reshape(1, D).astype(bf),
        "wo_r": np.ascontiguousarray(
            wo.reshape(H // 2, 2, HD, D).transpose(1, 2, 0, 3).reshape(
                128, H // 2, D)).astype(bf),
        "bo_row": np.asarray(inputs["bo"], f32).reshape(1, D).astype(bf),
        "w1_r": w1_f.astype(bf),
        "b1_r": np.ascontiguousarray(b1_f.reshape(DFF // 128, 128).T),
        "la1_r": la1_f.astype(bf),
        "bt1": bt1.reshape(R, 1).astype(f32),
        "lb1_r": lb1.astype(bf),
        "w2_r": w2.astype(bf),
        "b2_row": np.asarray(inputs["b2"], f32).reshape(1, D).astype(bf),
        "la2_r": la2.astype(bf),
        "lb2_r": lb2.astype(bf),
    }
    return common


def kernel(**inputs) -> np.ndarray:
    nbt = int(np.asarray(inputs["num_backbone_tokens"]))
    if nbt not in _cache:
        _cache[nbt] = _build(nbt)
    nc = _cache[nbt]

    common = _prep_host(inputs)
    x = np.asarray(inputs["x"], np.float32)
    in_maps = [dict(common, x=np.ascontiguousarray(x[b]))
               for b in range(N_CORES)]
    profile = os.environ.get("KERNEL_PROFILE") == "1"
    if profile:
        _install_ntff_hook()
    res = run_bass_kernel_spmd(nc, in_maps, core_ids=list(range(N_CORES)),
                               trace=profile)
    if res.exec_time_ns is not None:
        global last_exec_time_ns
        last_exec_time_ns = res.exec_time_ns
    return np.stack([res.results[b]["out"] for b in range(N_CORES)], axis=0)
